# revision 15
# baseline (speedup 1.0000x reference)
"""Trainium2 Bass kernel for nn_ContextualViewModel_48833778155979.

Computation (see reference):
    station_feats = x[sx, sy]            # (K, F) gather -- on host (replicated)
    y = station_feats @ W                # (K, F) tiny matmul -- on device, fp32
    res[h, w, :] = sum_k d[h, w, k] * y[k, :]   # big (H*W, K) @ (K, F) matmul

Sharding: H axis split across 8 cores (48 rows each -> 18432 grid cells/core).

Device strategy per core (v6 -- transpose-free, bf16 I/O, double-buffered
PSUM, fast startup):
  - The host ships d pre-transposed per core as d_t = (K, 18432) bf16, so the
    contraction dim k is already on SBUF partitions: no on-device transposes.
  - Startup: a burst of throwaway matmuls on memset scratch (no data
    dependencies) warms the PE HAM clock gate while the first DMAs fly.
    The two constant matrices arrive as ONE host-packed 512 KiB DMA laid out
    exactly as SBUF wants it (no rearrange, 4 KiB per partition line).
  - y = (x[sx,sy]) @ W is computed on device with precise fp32 matmuls and
    cast to bf16; its four 128x128 chunks are the stationary operands.
  - Main loop: per 2048-column block, two 1024-column half-blocks; per
    (half-block, f-half) four matmuls of N=512 accumulate over the two
    128-wide k chunks into a [128, 1024] fp32 PSUM tile (2 banks). Tiles are
    keyed by (f-half, half-block parity): 4 tags x 2 banks = all 8 banks,
    giving true double buffering.
  - PSUM -> SBUF copies cast to bf16: vector engine takes f-half 0, scalar
    engine f-half 1. One 512 KiB output DMA per half-block on the scalar
    HWDGE ring (earlier drain, shorter tail); input DMAs (256 KiB per
    k-chunk half-block) ride the sync HWDGE ring.
  - The host casts the (F, 18432) bf16 shards up to fp32 and transposes back.

bf16 rounding of d, y and out adds ~3e-3 relative error (tolerance 1e-2).
"""

import sys

sys.path.insert(0, "/opt/trn_rl_repo")

from contextlib import ExitStack

import ml_dtypes
import numpy as np

import concourse.bacc as bacc
import concourse.mybir as mybir
import concourse.tile as tile
from concourse.bass_utils import run_bass_kernel_spmd

H, WG, F = 384, 384, 256
K = 256
NCORES = 8
HS = H // NCORES          # 48 grid rows per core
ROWS = HS * WG            # 18432 cells per core
BLK = 2048                # output columns per block
NBLK = ROWS // BLK        # 9
HB = 1024                 # half-block columns (one PSUM tile / output DMA)
WARMUP_MM = 18            # dummy N=256 matmuls to warm the PE HAM

F32 = mybir.dt.float32
BF16 = mybir.dt.bfloat16

_cache: dict = {}
last_results = None  # BassKernelResults of the most recent kernel() call


def _build_program():
    if "nc" in _cache:
        return _cache["nc"]

    nc = bacc.Bacc(
        "TRN2", target_bir_lowering=False, debug=False, num_devices=NCORES
    )

    dt_ext = nc.dram_tensor("d_t", [K, ROWS], BF16, kind="ExternalInput").ap()
    # Host-packed constants: [cp, cc, 0, :] = station_t chunk, [cp, cc, 1, :] = W chunk
    cst_ext = nc.dram_tensor(
        "const_pack", [128, 2, 2, K], F32, kind="ExternalInput"
    ).ap()
    out_ext = nc.dram_tensor("out_t", [F, ROWS], BF16, kind="ExternalOutput").ap()

    with tile.TileContext(nc) as tc, ExitStack() as ctx:
        const = ctx.enter_context(tc.tile_pool(name="const", bufs=1))
        dpool = ctx.enter_context(tc.tile_pool(name="din", bufs=8))
        opool = ctx.enter_context(tc.tile_pool(name="dout", bufs=4))
        # 4 tags x 1 buf x [128, 1024] f32 = 2 banks each = all 8 PSUM banks
        mpsum = ctx.enter_context(tc.tile_pool(name="mpsum", bufs=1, space="PSUM"))

        # --- PE warm-up on memset scratch: zero data deps, starts at t=0 ----
        wa = const.tile([128, 128], BF16)
        wb = const.tile([128, 256], BF16)
        nc.vector.memset(wa[:, :], 0.0)
        nc.vector.memset(wb[:, :], 0.0)
        wps = mpsum.tile([128, 1024], F32, name="wps", tag="q11")
        for _ in range(WARMUP_MM):
            nc.tensor.matmul(wps[:, :F], wa[:, :], wb[:, :], start=True, stop=True)

        # --- constants: one packed 512 KiB DMA on the scalar HWDGE ring -----
        cst = const.tile([128, 2, 2, K], F32)
        nc.scalar.dma_start(cst[:, :, :, :], cst_ext)

        # --- y = station_feats @ W (precise fp32), k-major, cast to bf16 ----
        y_sb = const.tile([128, 2, F], BF16)
        for kc in range(2):
            yps = mpsum.tile([128, 1024], F32, name=f"yps{kc}", tag=f"q0{kc}")
            for cc in range(2):
                nc.tensor.matmul(
                    yps[:, :F],
                    cst[:, cc, 0, kc * 128 : (kc + 1) * 128],
                    cst[:, cc, 1, :],
                    start=(cc == 0),
                    stop=(cc == 1),
                )
            nc.vector.tensor_copy(y_sb[:, kc, :], yps[:, :F])

        # --- main loop: out^T[f, m] = sum_k y[k, f] * d_t[k, m] -------------
        for b in range(NBLK):
            m0 = b * BLK
            din = [[None, None], [None, None]]  # [kc][hb]
            for hb in range(2):
                for kc in range(2):
                    t = dpool.tile([128, HB], BF16, name=f"din{kc}", tag=f"din{kc}")
                    c0 = m0 + hb * HB
                    nc.sync.dma_start(
                        t[:, :], dt_ext[kc * 128 : (kc + 1) * 128, c0 : c0 + HB]
                    )
                    din[kc][hb] = t
            for hb in range(2):
                dout = opool.tile(
                    [128, 2, HB], BF16, name=f"do{hb}", tag=f"do{hb}"
                )
                for fh in range(2):
                    q = mpsum.tile(
                        [128, 1024], F32, name=f"q{fh}{hb % 2}", tag=f"q{fh}{hb % 2}"
                    )
                    for kc in range(2):
                        for sb in range(2):
                            nc.tensor.matmul(
                                q[:, sb * 512 : (sb + 1) * 512],
                                y_sb[:, kc, fh * 128 : (fh + 1) * 128],
                                din[kc][hb][:, sb * 512 : (sb + 1) * 512],
                                start=(kc == 0),
                                stop=(kc == 1),
                            )
                    copy = nc.vector.tensor_copy if fh == 0 else nc.scalar.copy
                    copy(dout[:, fh, :], q[:, :])
                c0 = m0 + hb * HB
                nc.scalar.dma_start(
                    out_ext[:, c0 : c0 + HB].rearrange("(fc fp) m -> fp fc m", fc=2),
                    dout[:, :, :],
                )

    nc.compile()
    _cache["nc"] = nc
    return nc


def kernel(x, d, W, sx, sy):
    x = np.asarray(x, dtype=np.float32)
    d = np.asarray(d, dtype=np.float32)
    W = np.asarray(W, dtype=np.float32)
    sx = np.asarray(sx, dtype=np.int32)
    sy = np.asarray(sy, dtype=np.int32)

    # Host-side gather of the K station feature vectors (replicated to all
    # cores, per the sharding strategy), pre-transposed to contraction-major,
    # packed together with W in the exact SBUF layout (one DMA, no rearrange).
    station_t = x[sx, sy].T                      # (c, k)
    cst = np.empty((128, 2, 2, K), dtype=np.float32)
    for cc in range(2):
        cst[:, cc, 0, :] = station_t[cc * 128 : (cc + 1) * 128, :]
        cst[:, cc, 1, :] = W[cc * 128 : (cc + 1) * 128, :]
    bf16 = ml_dtypes.bfloat16

    nc = _build_program()

    dd = d.reshape(NCORES, ROWS, K)
    in_maps = []
    for c in range(NCORES):
        in_maps.append(
            {
                "d_t": dd[c].T.astype(bf16),  # (K, ROWS) contraction-major
                "const_pack": cst,
            }
        )

    res = run_bass_kernel_spmd(nc, in_maps, list(range(NCORES)))
    global last_results
    last_results = res
    out = np.concatenate(
        [
            np.asarray(r["out_t"]).astype(np.float32).T.reshape(HS, WG, F)
            for r in res.results
        ],
        axis=0,
    )
    return out


if __name__ == "__main__":
    rng = np.random.default_rng(0)
    x = rng.standard_normal((H, WG, F), dtype=np.float32)
    d = rng.random((H, WG, K), dtype=np.float32)
    W = rng.standard_normal((K, F), dtype=np.float32) / np.sqrt(F)
    sx = rng.integers(0, H, size=(K,)).astype(np.int32)
    sy = rng.integers(0, WG, size=(K,)).astype(np.int32)
    out = kernel(x, d, W, sx, sy)
    y = x[sx, sy].astype(np.float64) @ W.astype(np.float64)
    exp = d.reshape(-1, K).astype(np.float64) @ y
    exp = exp.reshape(H, WG, F)
    err = np.linalg.norm(out - exp) / np.linalg.norm(exp)
    print("rel err:", err)


# revision 18
# speedup vs baseline: 1.0828x; 1.0828x over previous
"""Trainium2 Bass kernel for nn_ContextualViewModel_48833778155979.

Computation (see reference):
    station_feats = x[sx, sy]            # (K, F) gather -- on host (replicated)
    y = station_feats @ W                # (K, F) tiny matmul -- on device, fp32
    res[h, w, :] = sum_k d[h, w, k] * y[k, :]   # big (H*W, K) @ (K, F) matmul

Sharding: H axis split across 8 cores (48 rows each -> 18432 grid cells/core).

Device strategy per core (v6 -- transpose-free, bf16 I/O, double-buffered
PSUM, fast startup):
  - The host ships d pre-transposed per core as d_t = (K, 18432) bf16, so the
    contraction dim k is already on SBUF partitions: no on-device transposes.
  - Startup: a burst of throwaway matmuls on memset scratch (no data
    dependencies) warms the PE HAM clock gate while the first DMAs fly.
    The two constant matrices arrive as ONE host-packed 512 KiB DMA laid out
    exactly as SBUF wants it (no rearrange, 4 KiB per partition line).
  - y = (x[sx,sy]) @ W is computed on device with precise fp32 matmuls and
    cast to bf16; its four 128x128 chunks are the stationary operands.
  - Main loop: per 2048-column block, two 1024-column half-blocks; per
    (half-block, f-half) four matmuls of N=512 accumulate over the two
    128-wide k chunks into a [128, 1024] fp32 PSUM tile (2 banks). Tiles are
    keyed by (f-half, half-block parity): 4 tags x 2 banks = all 8 banks,
    giving true double buffering.
  - PSUM -> SBUF copies cast to bf16: vector engine takes f-half 0, scalar
    engine f-half 1. One 512 KiB output DMA per half-block on the scalar
    HWDGE ring (earlier drain, shorter tail); input DMAs (256 KiB per
    k-chunk half-block) ride the sync HWDGE ring.
  - The host casts the (F, 18432) bf16 shards up to fp32 and transposes back.

bf16 rounding of d, y and out adds ~3e-3 relative error (tolerance 1e-2).
"""

import sys

sys.path.insert(0, "/opt/trn_rl_repo")

from contextlib import ExitStack

import ml_dtypes
import numpy as np

import concourse.bacc as bacc
import concourse.mybir as mybir
import concourse.tile as tile
from concourse.bass_utils import run_bass_kernel_spmd

H, WG, F = 384, 384, 256
K = 256
NCORES = 8
HS = H // NCORES          # 48 grid rows per core
ROWS = HS * WG            # 18432 cells per core
BLK = 2048                # output columns per block
NBLK = ROWS // BLK        # 9
HB = 1024                 # half-block columns (one PSUM tile / output DMA)
WARMUP_MM = 24            # dummy N=256 matmuls to warm the PE HAM

F32 = mybir.dt.float32
BF16 = mybir.dt.bfloat16

_cache: dict = {}
last_results = None  # BassKernelResults of the most recent kernel() call


def _build_program():
    if "nc" in _cache:
        return _cache["nc"]

    nc = bacc.Bacc(
        "TRN2", target_bir_lowering=False, debug=False, num_devices=NCORES
    )

    dt_ext = nc.dram_tensor("d_t", [K, ROWS], BF16, kind="ExternalInput").ap()
    # Host-packed constants: [cp, cc, 0, :] = station_t chunk, [cp, cc, 1, :] = W chunk
    cst_ext = nc.dram_tensor(
        "const_pack", [128, 2, 2, K], F32, kind="ExternalInput"
    ).ap()
    out_ext = nc.dram_tensor("out_t", [F, ROWS], BF16, kind="ExternalOutput").ap()

    with tile.TileContext(nc) as tc, ExitStack() as ctx:
        const = ctx.enter_context(tc.tile_pool(name="const", bufs=1))
        dpool = ctx.enter_context(tc.tile_pool(name="din", bufs=8))
        opool = ctx.enter_context(tc.tile_pool(name="dout", bufs=3))
        # 4 tags x 1 buf x [128, 1024] f32 = 2 banks each = all 8 PSUM banks
        mpsum = ctx.enter_context(tc.tile_pool(name="mpsum", bufs=1, space="PSUM"))

        # --- PE warm-up on memset scratch: zero data deps, starts at t=0 ----
        wa = const.tile([128, 128], BF16)
        wb = const.tile([128, 256], BF16)
        nc.vector.memset(wa[:, :], 0.0)
        nc.vector.memset(wb[:, :], 0.0)
        wps = mpsum.tile([128, 1024], F32, name="wps", tag="q11")
        for _ in range(WARMUP_MM):
            nc.tensor.matmul(wps[:, :F], wa[:, :], wb[:, :], start=True, stop=True)

        # --- constants: one packed 512 KiB DMA on the scalar HWDGE ring -----
        cst = const.tile([128, 2, 2, K], F32)
        nc.scalar.dma_start(cst[:, :, :, :], cst_ext)

        # --- y = station_feats @ W (precise fp32), k-major, cast to bf16 ----
        y_sb = const.tile([128, 2, F], BF16)
        for kc in range(2):
            yps = mpsum.tile([128, 1024], F32, name=f"yps{kc}", tag=f"q0{kc}")
            for cc in range(2):
                nc.tensor.matmul(
                    yps[:, :F],
                    cst[:, cc, 0, kc * 128 : (kc + 1) * 128],
                    cst[:, cc, 1, :],
                    start=(cc == 0),
                    stop=(cc == 1),
                )
            nc.vector.tensor_copy(y_sb[:, kc, :], yps[:, :F])

        # --- main loop: out^T[f, m] = sum_k y[k, f] * d_t[k, m] -------------
        for b in range(NBLK):
            m0 = b * BLK
            din = [[None, None], [None, None]]  # [kc][hb]
            for hb in range(2):
                for kc in range(2):
                    t = dpool.tile([128, HB], BF16, name=f"din{kc}", tag=f"din{kc}")
                    c0 = m0 + hb * HB
                    nc.sync.dma_start(
                        t[:, :], dt_ext[kc * 128 : (kc + 1) * 128, c0 : c0 + HB]
                    )
                    din[kc][hb] = t
            dout = opool.tile([128, 2, BLK], BF16, tag="dout")
            for hb in range(2):
                for fh in range(2):
                    q = mpsum.tile(
                        [128, 1024], F32, name=f"q{fh}{hb % 2}", tag=f"q{fh}{hb % 2}"
                    )
                    for kc in range(2):
                        for sb in range(2):
                            nc.tensor.matmul(
                                q[:, sb * 512 : (sb + 1) * 512],
                                y_sb[:, kc, fh * 128 : (fh + 1) * 128],
                                din[kc][hb][:, sb * 512 : (sb + 1) * 512],
                                start=(kc == 0),
                                stop=(kc == 1),
                            )
                    copy = nc.vector.tensor_copy if fh == 0 else nc.scalar.copy
                    copy(dout[:, fh, hb * HB : (hb + 1) * HB], q[:, :])
                c0 = m0 + hb * HB
                nc.scalar.dma_start(
                    out_ext[:, c0 : c0 + HB].rearrange("(fc fp) m -> fp fc m", fc=2),
                    dout[:, :, hb * HB : (hb + 1) * HB],
                )

    nc.compile()
    _cache["nc"] = nc
    return nc


def kernel(x, d, W, sx, sy):
    x = np.asarray(x, dtype=np.float32)
    d = np.asarray(d, dtype=np.float32)
    W = np.asarray(W, dtype=np.float32)
    sx = np.asarray(sx, dtype=np.int32)
    sy = np.asarray(sy, dtype=np.int32)

    # Host-side gather of the K station feature vectors (replicated to all
    # cores, per the sharding strategy), pre-transposed to contraction-major,
    # packed together with W in the exact SBUF layout (one DMA, no rearrange).
    station_t = x[sx, sy].T                      # (c, k)
    cst = np.empty((128, 2, 2, K), dtype=np.float32)
    for cc in range(2):
        cst[:, cc, 0, :] = station_t[cc * 128 : (cc + 1) * 128, :]
        cst[:, cc, 1, :] = W[cc * 128 : (cc + 1) * 128, :]
    bf16 = ml_dtypes.bfloat16

    nc = _build_program()

    dd = d.reshape(NCORES, ROWS, K)
    in_maps = []
    for c in range(NCORES):
        in_maps.append(
            {
                "d_t": dd[c].T.astype(bf16),  # (K, ROWS) contraction-major
                "const_pack": cst,
            }
        )

    res = run_bass_kernel_spmd(nc, in_maps, list(range(NCORES)))
    global last_results
    last_results = res
    out = np.concatenate(
        [
            np.asarray(r["out_t"]).astype(np.float32).T.reshape(HS, WG, F)
            for r in res.results
        ],
        axis=0,
    )
    return out


if __name__ == "__main__":
    rng = np.random.default_rng(0)
    x = rng.standard_normal((H, WG, F), dtype=np.float32)
    d = rng.random((H, WG, K), dtype=np.float32)
    W = rng.standard_normal((K, F), dtype=np.float32) / np.sqrt(F)
    sx = rng.integers(0, H, size=(K,)).astype(np.int32)
    sy = rng.integers(0, WG, size=(K,)).astype(np.int32)
    out = kernel(x, d, W, sx, sy)
    y = x[sx, sy].astype(np.float64) @ W.astype(np.float64)
    exp = d.reshape(-1, K).astype(np.float64) @ y
    exp = exp.reshape(H, WG, F)
    err = np.linalg.norm(out - exp) / np.linalg.norm(exp)
    print("rel err:", err)
